# revision 10
# baseline (speedup 1.0000x reference)
"""Trainium2 Bass kernel for the bilinear classifier problem.

Reference computation (single full-shape op):
    xW     = x @ W                     # [512, 512]
    scores = xW @ embedding_matrix.T   # [512, 20000]

Sharding (classifier / tensor parallel over num_class, per sharding hint):
  - embedding_matrix (and output scores) sharded over num_class across 8 cores
  - x, W replicated on every core
  - no collectives: each core writes its score shard, host concatenates

Per-core device kernel (TensorE convention: out[M,N] = lhsT[K,M]^T @ rhs[K,N]):
  GEMM1: xWT[w, b] = sum_i W[i,w] * xT[i,b]    lhsT=W [1024,512], rhs=xT [1024,512]
  GEMM2: out[b, c] = sum_w xWT[w,b] * ET[w,c]  lhsT=xWT [512,512], rhs=ET_shard [512,2560]

Matmuls run in bf16 (inputs cast f32->bf16 inside the input DMA via the
gpsimd software-DGE queue; fp32 accumulate in PSUM; f32 output).

Pipelining: W/xT stream in per-k-subtile blocks; GEMM1 runs k-outer with
4 live PSUM banks so it chases the input DMA. A burst of dummy matmuls
during the input stream warms the PE clock gate (HAM) so the real GEMM
stream runs at 2.4 GHz from its first instruction.

Host-side prep (layout only, no reference FLOPs): inputs are pre-arranged
into SBUF-blocked layout [128 partitions, ...] so every DMA descriptor
reads one long contiguous run per partition:
  wx  [128, KO1, WORD_D + B]         per-k-subtile W_k || xT_k blocks
  ET  [128, N_CHUNKS, KO2, N_TILE]   E^T column-chunked + k-subtile-blocked
num_class is zero-padded 20000 -> 20480 so each core gets 2560 columns.
"""

import os

import numpy as np

B = 512
IMG_D = 1024
WORD_D = 512
NUM_CLASS = 20000
N_CORES = 8
C_PAD = 20480  # next multiple of 8*512
C_SHARD = C_PAD // N_CORES  # 2560
N_TILE = 512
N_CHUNKS = C_SHARD // N_TILE  # 5

KO1 = IMG_D // 128  # 8 k-subtiles for GEMM1
KO2 = WORD_D // 128  # 4 k-subtiles for GEMM2
MW = WORD_D // 128  # 4 m-subtiles of xWT
MB = B // 128  # 4 m-subtiles of scores

WARMUP_MM = 48  # dummy matmuls to flip the PE HAM clock gate to 2.4 GHz

_CACHE = {}


def _build_nc():
    import concourse.mybir as mybir
    import concourse.tile as tile
    from concourse import bacc

    f32 = mybir.dt.float32
    bf16 = mybir.dt.bfloat16

    nc = bacc.Bacc(None, target_bir_lowering=False, debug=False)

    wx_ext = nc.declare_dram_parameter(
        "wx", [128, KO1, WORD_D + B], f32, isOutput=False
    )
    ET_ext = nc.declare_dram_parameter(
        "ET", [128, N_CHUNKS, KO2, N_TILE], f32, isOutput=False
    )
    out_ext = nc.declare_dram_parameter("out", [B, C_SHARD], f32, isOutput=True)

    with tile.TileContext(nc) as tc:
        with (
            tc.tile_pool(name="const", bufs=1) as cpool,
            tc.tile_pool(name="outp", bufs=6) as opool,
            tc.tile_pool(name="ps1", bufs=1, space="PSUM") as ps1,
            tc.tile_pool(name="ps2", bufs=3, space="PSUM") as ps2,
        ):
            # --- inputs: gpsimd software-DGE casting DMA, f32 -> bf16 SBUF.
            # Dedicated tiles (no pool recycling): streaming never stalls.
            wx_sb = []
            for k in range(KO1):
                t = cpool.tile([128, WORD_D + B], bf16, name=f"wx{k}")
                nc.gpsimd.dma_start(t[:], wx_ext[:, k])
                wx_sb.append(t)
            et_sb = []
            for n in range(N_CHUNKS):
                t = cpool.tile([128, KO2, N_TILE], bf16, name=f"et{n}")
                nc.gpsimd.dma_start(t[:], ET_ext[:, n])
                et_sb.append(t)

            # --- PE warmup: dummy matmuls on a zeroed tile, no data deps.
            # They run while inputs stream in, flipping HAM to 8/8 before
            # the real GEMM chain starts.
            dummy = cpool.tile([128, 64], bf16, name="dummy")
            nc.vector.memset(dummy[:], 0.0)
            wps = ps1.tile([128, 64], f32, tag="warmps")
            for _ in range(WARMUP_MM):
                nc.tensor.matmul(
                    wps[:64, :], lhsT=dummy[:], rhs=dummy[:], start=True, stop=True
                )

            # --- GEMM1: xWT [512(w), 512(b)], k-outer so it chases the DMA ---
            g1ps = [
                ps1.tile([128, B], f32, tag=f"g1psum{mw}", name=f"g1psum{mw}")
                for mw in range(MW)
            ]
            for k in range(KO1):
                for mw in range(MW):
                    nc.tensor.matmul(
                        g1ps[mw][:],
                        lhsT=wx_sb[k][:, mw * 128 : (mw + 1) * 128],
                        rhs=wx_sb[k][:, WORD_D:],
                        start=(k == 0),
                        stop=(k == KO1 - 1),
                    )
            xwt_sb = cpool.tile([128, MW, B], bf16, name="xwt")
            for mw in range(MW):
                if mw % 2 == 0:
                    nc.vector.tensor_copy(out=xwt_sb[:, mw, :], in_=g1ps[mw][:])
                else:
                    nc.scalar.copy(out=xwt_sb[:, mw, :], in_=g1ps[mw][:])

            # --- GEMM2: out [512(b), 2560(c)] in column chunks of 512 ---
            for n in range(N_CHUNKS):
                for mb in range(MB):
                    ps = ps2.tile([128, N_TILE], f32, tag="g2psum")
                    for k in range(KO2):
                        nc.tensor.matmul(
                            ps[:],
                            lhsT=xwt_sb[:, k, mb * 128 : (mb + 1) * 128],
                            rhs=et_sb[n][:, k, :],
                            start=(k == 0),
                            stop=(k == KO2 - 1),
                        )
                    o_sb = opool.tile([128, N_TILE], f32, tag="osb")
                    # split PSUM evictions across DVE and ACT so neither
                    # engine serializes the pipeline
                    if (n * MB + mb) % 2 == 0:
                        nc.vector.tensor_copy(out=o_sb[:], in_=ps[:])
                    else:
                        nc.scalar.copy(out=o_sb[:], in_=ps[:])
                    nc.sync.dma_start(
                        out_ext[mb * 128 : (mb + 1) * 128, n * N_TILE : (n + 1) * N_TILE],
                        o_sb[:],
                    )

    nc.compile()
    return nc


def _get_nc():
    if "nc" not in _CACHE:
        _CACHE["nc"] = _build_nc()
    return _CACHE["nc"]


def _prep_host(x, embedding_matrix, W):
    """Blocked input layouts. Pure layout transforms (transpose/pad/reshape)."""
    x = np.asarray(x, dtype=np.float32)
    W = np.asarray(W, dtype=np.float32)
    E = np.asarray(embedding_matrix, dtype=np.float32)

    # W [IMG_D, WORD_D] -> [128, KO1, WORD_D];  xT [IMG_D, B] -> [128, KO1, B]
    W_blk = W.reshape(KO1, 128, WORD_D).transpose(1, 0, 2)
    xT_blk = x.T.reshape(KO1, 128, B).transpose(1, 0, 2)
    wx = np.ascontiguousarray(np.concatenate([W_blk, xT_blk], axis=2))

    # ET [WORD_D, C_PAD] -> [128, N_CORES, N_CHUNKS, KO2, N_TILE]
    ET = np.zeros((WORD_D, C_PAD), dtype=np.float32)
    ET[:, :NUM_CLASS] = E.T
    # w = ko*128 + p ; c_global = core*C_SHARD + n*N_TILE + cc
    ET_blk = ET.reshape(KO2, 128, N_CORES, N_CHUNKS, N_TILE).transpose(1, 2, 3, 0, 4)
    ET_blk = np.ascontiguousarray(ET_blk)
    return wx, ET_blk


def kernel(x: np.ndarray, embedding_matrix: np.ndarray, W: np.ndarray) -> np.ndarray:
    from concourse.bass_utils import run_bass_kernel_spmd

    trace = os.environ.get("KERNEL_TRACE", "0") == "1"

    wx, ET_blk = _prep_host(x, embedding_matrix, W)

    in_maps = [
        {"wx": wx, "ET": np.ascontiguousarray(ET_blk[:, c])} for c in range(N_CORES)
    ]

    nc = _get_nc()
    tmpdir = os.environ.get("KERNEL_TRACE_DIR") if trace else None
    if tmpdir:
        os.makedirs(tmpdir, exist_ok=True)
    res = run_bass_kernel_spmd(
        nc, in_maps, core_ids=list(range(N_CORES)), trace=trace, tmpdir=tmpdir
    )
    if trace:
        _CACHE["last_exec_time_ns"] = res.exec_time_ns
        _CACHE["last_trace"] = res.instructions_and_trace

    full = np.concatenate([res.results[c]["out"] for c in range(N_CORES)], axis=1)
    return np.ascontiguousarray(full[:, :NUM_CLASS])


# revision 12
# speedup vs baseline: 1.1684x; 1.1684x over previous
"""Trainium2 Bass kernel for the bilinear classifier problem.

Reference computation (single full-shape op):
    xW     = x @ W                     # [512, 512]
    scores = xW @ embedding_matrix.T   # [512, 20000]

Sharding (classifier / tensor parallel over num_class, per sharding hint):
  - embedding_matrix (and output scores) sharded over num_class across 8 cores
  - x, W replicated on every core
  - no collectives: each core writes its score shard, host concatenates

Per-core device kernel (TensorE convention: out[M,N] = lhsT[K,M]^T @ rhs[K,N]):
  GEMM1: xWT[w, b] = sum_i W[i,w] * xT[i,b]    lhsT=W [1024,512], rhs=xT [1024,512]
  GEMM2: out[b, c] = sum_w xWT[w,b] * ET[w,c]  lhsT=xWT [512,512], rhs=ET_shard [512,2560]

Matmuls run in bf16 (inputs cast f32->bf16 inside the input DMA via the
gpsimd software-DGE queue; fp32 accumulate in PSUM; f32 output).

Pipelining: W/xT stream in per-k-subtile blocks; GEMM1 runs k-outer with
4 live PSUM banks so it chases the input DMA. A burst of dummy matmuls
during the input stream warms the PE clock gate (HAM) so the real GEMM
stream runs at 2.4 GHz from its first instruction.

Host-side prep (layout only, no reference FLOPs): inputs are pre-arranged
into SBUF-blocked layout [128 partitions, ...] so every DMA descriptor
reads one long contiguous run per partition:
  wx  [128, KO1, WORD_D + B]         per-k-subtile W_k || xT_k blocks
  ET  [128, N_CHUNKS, KO2, N_TILE]   E^T column-chunked + k-subtile-blocked
num_class is zero-padded 20000 -> 20480 so each core gets 2560 columns.
"""

import os

import numpy as np

B = 512
IMG_D = 1024
WORD_D = 512
NUM_CLASS = 20000
N_CORES = 8
C_PAD = 20480  # next multiple of 8*512
C_SHARD = C_PAD // N_CORES  # 2560
N_TILE = 512
N_CHUNKS = C_SHARD // N_TILE  # 5

KO1 = IMG_D // 128  # 8 k-subtiles for GEMM1
KO2 = WORD_D // 128  # 4 k-subtiles for GEMM2
MW = WORD_D // 128  # 4 m-subtiles of xWT
MB = B // 128  # 4 m-subtiles of scores

WARMUP_MM = 48  # dummy matmuls to flip the PE HAM clock gate to 2.4 GHz

_CACHE = {}


def _build_nc():
    import concourse.mybir as mybir
    import concourse.tile as tile
    from concourse import bacc

    f32 = mybir.dt.float32
    bf16 = mybir.dt.bfloat16

    nc = bacc.Bacc(None, target_bir_lowering=False, debug=False)

    wx_ext = nc.declare_dram_parameter(
        "wx", [128, KO1, WORD_D + B], f32, isOutput=False
    )
    ET_ext = nc.declare_dram_parameter(
        "ET", [128, N_CHUNKS, KO2, N_TILE], f32, isOutput=False
    )
    out_ext = nc.declare_dram_parameter("out", [B, C_SHARD], f32, isOutput=True)

    with tile.TileContext(nc) as tc:
        with (
            tc.tile_pool(name="const", bufs=1) as cpool,
            tc.tile_pool(name="outp", bufs=8) as opool,
            tc.tile_pool(name="ps1", bufs=1, space="PSUM") as ps1,
            tc.tile_pool(name="ps2", bufs=4, space="PSUM") as ps2,
        ):
            # --- inputs: gpsimd software-DGE casting DMA, f32 -> bf16 SBUF.
            # Dedicated tiles (no pool recycling): streaming never stalls.
            wx_sb = []
            for k in range(KO1):
                t = cpool.tile([128, WORD_D + B], bf16, name=f"wx{k}")
                nc.gpsimd.dma_start(t[:], wx_ext[:, k])
                wx_sb.append(t)
            et_sb = []
            for n in range(N_CHUNKS):
                t = cpool.tile([128, KO2, N_TILE], bf16, name=f"et{n}")
                nc.gpsimd.dma_start(t[:], ET_ext[:, n])
                et_sb.append(t)

            # --- PE warmup: dummy matmuls on a zeroed tile, no data deps.
            # They run while inputs stream in, flipping HAM to 8/8 before
            # the real GEMM chain starts.
            dummy = cpool.tile([128, 64], bf16, name="dummy")
            nc.vector.memset(dummy[:], 0.0)
            # share the g2psum tag so the warmup bank is recycled by GEMM2
            wps = ps2.tile([128, N_TILE], f32, tag="g2psum", name="warmps")
            for _ in range(WARMUP_MM):
                nc.tensor.matmul(
                    wps[:64, :64], lhsT=dummy[:], rhs=dummy[:], start=True, stop=True
                )

            # --- GEMM1: xWT [512(w), 512(b)], k-outer so it chases the DMA ---
            g1ps = [
                ps1.tile([128, B], f32, tag=f"g1psum{mw}", name=f"g1psum{mw}")
                for mw in range(MW)
            ]
            for k in range(KO1):
                for mw in range(MW):
                    nc.tensor.matmul(
                        g1ps[mw][:],
                        lhsT=wx_sb[k][:, mw * 128 : (mw + 1) * 128],
                        rhs=wx_sb[k][:, WORD_D:],
                        start=(k == 0),
                        stop=(k == KO1 - 1),
                    )
            xwt_sb = [
                cpool.tile([128, B], bf16, name=f"xwt{mw}") for mw in range(MW)
            ]
            for mw in range(MW):
                if mw % 2 == 0:
                    nc.vector.tensor_copy(out=xwt_sb[mw][:], in_=g1ps[mw][:])
                else:
                    nc.scalar.copy(out=xwt_sb[mw][:], in_=g1ps[mw][:])

            # --- GEMM2: out [512(b), 2560(c)] in column chunks of 512 ---
            for n in range(N_CHUNKS):
                for mb in range(MB):
                    ps = ps2.tile([128, N_TILE], f32, tag="g2psum")
                    for k in range(KO2):
                        nc.tensor.matmul(
                            ps[:],
                            lhsT=xwt_sb[k][:, mb * 128 : (mb + 1) * 128],
                            rhs=et_sb[n][:, k, :],
                            start=(k == 0),
                            stop=(k == KO2 - 1),
                        )
                    o_sb = opool.tile([128, N_TILE], f32, tag="osb")
                    # split PSUM evictions across DVE and ACT so neither
                    # engine serializes the pipeline
                    if (n * MB + mb) % 2 == 0:
                        nc.vector.tensor_copy(out=o_sb[:], in_=ps[:])
                    else:
                        nc.scalar.copy(out=o_sb[:], in_=ps[:])
                    nc.sync.dma_start(
                        out_ext[mb * 128 : (mb + 1) * 128, n * N_TILE : (n + 1) * N_TILE],
                        o_sb[:],
                    )

    nc.compile()
    return nc


def _get_nc():
    if "nc" not in _CACHE:
        _CACHE["nc"] = _build_nc()
    return _CACHE["nc"]


def _prep_host(x, embedding_matrix, W):
    """Blocked input layouts. Pure layout transforms (transpose/pad/reshape)."""
    x = np.asarray(x, dtype=np.float32)
    W = np.asarray(W, dtype=np.float32)
    E = np.asarray(embedding_matrix, dtype=np.float32)

    # W [IMG_D, WORD_D] -> [128, KO1, WORD_D];  xT [IMG_D, B] -> [128, KO1, B]
    W_blk = W.reshape(KO1, 128, WORD_D).transpose(1, 0, 2)
    xT_blk = x.T.reshape(KO1, 128, B).transpose(1, 0, 2)
    wx = np.ascontiguousarray(np.concatenate([W_blk, xT_blk], axis=2))

    # ET [WORD_D, C_PAD] -> [128, N_CORES, N_CHUNKS, KO2, N_TILE]
    ET = np.zeros((WORD_D, C_PAD), dtype=np.float32)
    ET[:, :NUM_CLASS] = E.T
    # w = ko*128 + p ; c_global = core*C_SHARD + n*N_TILE + cc
    ET_blk = ET.reshape(KO2, 128, N_CORES, N_CHUNKS, N_TILE).transpose(1, 2, 3, 0, 4)
    ET_blk = np.ascontiguousarray(ET_blk)
    return wx, ET_blk


def kernel(x: np.ndarray, embedding_matrix: np.ndarray, W: np.ndarray) -> np.ndarray:
    from concourse.bass_utils import run_bass_kernel_spmd

    trace = os.environ.get("KERNEL_TRACE", "0") == "1"

    wx, ET_blk = _prep_host(x, embedding_matrix, W)

    in_maps = [
        {"wx": wx, "ET": np.ascontiguousarray(ET_blk[:, c])} for c in range(N_CORES)
    ]

    nc = _get_nc()
    tmpdir = os.environ.get("KERNEL_TRACE_DIR") if trace else None
    if tmpdir:
        os.makedirs(tmpdir, exist_ok=True)
    res = run_bass_kernel_spmd(
        nc, in_maps, core_ids=list(range(N_CORES)), trace=trace, tmpdir=tmpdir
    )
    if trace:
        _CACHE["last_exec_time_ns"] = res.exec_time_ns
        _CACHE["last_trace"] = res.instructions_and_trace

    full = np.concatenate([res.results[c]["out"] for c in range(N_CORES)], axis=1)
    return np.ascontiguousarray(full[:, :NUM_CLASS])
